# revision 11
# baseline (speedup 1.0000x reference)
"""Trainium2 Bass kernel for nn_MultiHeadMPSRecurrence.

Reference computation (B=4, T=8192, D=H=1024, fp32):
    ih    = x @ W_ih + b_ih
    gate  = sigmoid(x @ W_gate + b_gate)
    alpha = (1 - gate) * exp(log_a)          # elementwise, [B,T,H]
    beta  = gate * ih
    h_t   = alpha_t * h_{t-1} + beta_t       # scan over T, h_0 = 0
    hn    = LayerNorm_H(h) * ln_g + ln_b
    y     = (hn @ W_out + b_out) @ W_proj + b_proj

Sharding (8 cores, zero cross-core communication):
    core c -> batch b = c//2, time-half s = c%2 (4096 output rows each).
    The scan has alpha <= exp(log_a) = 0.905 always, so influence decays
    by >= 0.905^t; 256 warmup rows reconstruct the carry state to ~8e-12.
    Second-half cores prepend the real 256 preceding rows; first-half
    cores prepend zeros (x=0 -> beta=0 -> h stays 0, exact).

Host-side folds (pure input marshaling, fp64 then cast to fp32):
    a     = exp(log_a), na = -a
    W_c   = diag(ln_g) @ (W_out @ W_proj)         # single fused output matmul
    bc    = ln_b @ (W_out@W_proj) + b_out @ W_proj + b_proj
    s_row = colsum(W_c)
    y     = r * (h @ W_c) + (-r*mu) * s_row + bc  # LayerNorm applied post-matmul

On-chip layout: x is transposed on the host per shard -> xT [1024, 4352]
so every matmul has its contraction dim on partitions, and the scan runs
along the free (time) dim via the DVE tensor_tensor_scan instruction in
[channel-partition, time-free] layout. Matmuls use float32r (FP22 reads,
fp32 accumulate) which streams at 1 col/cycle for free dims >= 256.
"""

import os

os.environ.setdefault("MYCRO_LOCAL_CACHE", "1")

import numpy as np

B, T, D, H = 4, 8192, 1024, 1024
EPS = 1e-5
WARM = 256
SEG = T // 2            # 4096 output rows per core
RT = SEG + WARM         # 4352 input rows per core
C = 512                 # time-chunk size
P = 128
KD = D // P             # 8 contraction groups over d
KH = H // P             # 8 groups over h
TT = C // P             # 4 t-tiles per chunk
NCORES = 8

_CACHE = {}


def _build(rt, warm, nch):
    """Build + compile the Bass program for one core (SPMD across 8)."""
    from contextlib import ExitStack

    import concourse.bass as bass
    import concourse.tile as tile
    from concourse import bacc, mybir
    from concourse.bass import ts

    f32 = mybir.dt.float32
    f32r = mybir.dt.float32r
    AF = mybir.ActivationFunctionType
    OP = mybir.AluOpType

    nc = bacc.Bacc("TRN2", target_bir_lowering=False, debug=False)

    xT = nc.dram_tensor("xT", [D, rt], f32r, kind="ExternalInput").ap()
    wih = nc.dram_tensor("wih", [D, H], f32r, kind="ExternalInput").ap()
    wg = nc.dram_tensor("wg", [D, H], f32r, kind="ExternalInput").ap()
    wc = nc.dram_tensor("wc", [H, D], f32r, kind="ExternalInput").ap()
    # rows: 0=a, 1=-a, 2=b_ih, 3=b_gate, 4=colsum(W_c), 5=bc, 6=ones
    vecs = nc.dram_tensor("vecs", [7, H], f32, kind="ExternalInput").ap()
    y = nc.dram_tensor("y", [nch * C, D], f32, kind="ExternalOutput").ap()

    with ExitStack() as ctx:
        tc = ctx.enter_context(tile.TileContext(nc))
        wp = ctx.enter_context(tc.tile_pool(name="w", bufs=1))
        xtp = ctx.enter_context(tc.tile_pool(name="xt", bufs=12))
        iop = ctx.enter_context(tc.tile_pool(name="io", bufs=4))
        hp = ctx.enter_context(tc.tile_pool(name="h", bufs=2))
        hsqp = ctx.enter_context(tc.tile_pool(name="hsq", bufs=1))
        yp = ctx.enter_context(tc.tile_pool(name="y", bufs=3))
        rp = ctx.enter_context(tc.tile_pool(name="r", bufs=8))
        pproj = ctx.enter_context(tc.tile_pool(name="pproj", bufs=4, space="PSUM"))
        pout = ctx.enter_context(tc.tile_pool(name="pout", bufs=2, space="PSUM"))
        pst = ctx.enter_context(tc.tile_pool(name="pst", bufs=2, space="PSUM"))

        # ---- constants / weights (resident in SBUF)
        w_ih_sb = wp.tile([P, KD, H], f32r)
        nc.sync.dma_start(w_ih_sb[:], wih.rearrange("(k p) h -> p k h", p=P))
        w_g_sb = wp.tile([P, KD, H], f32r)
        nc.sync.dma_start(w_g_sb[:], wg.rearrange("(k p) h -> p k h", p=P))
        w_c_sb = wp.tile([P, KH, D], f32r)
        nc.sync.dma_start(w_c_sb[:], wc.rearrange("(k p) d -> p k d", p=P))
        vec_sb = wp.tile([P, 7, KH], f32)
        nc.sync.dma_start(vec_sb[:], vecs.rearrange("v (g p) -> p v g", p=P))
        srow = vecs[4, :]
        sbc = wp.tile([P, D], f32)
        nc.sync.dma_start(
            sbc[:], bass.AP(tensor=srow.tensor, offset=srow.offset, ap=[[0, P]] + list(srow.ap))
        )
        bcrow = vecs[5, :]
        bcb = wp.tile([P, D], f32)
        nc.sync.dma_start(
            bcb[:], bass.AP(tensor=bcrow.tensor, offset=bcrow.offset, ap=[[0, P]] + list(bcrow.ap))
        )
        onesrow = vecs[6, :]
        # f32r matmuls require out free dim >= 2, so the ones rhs is 2 wide
        ones_sb = wp.tile([P, 2], f32r)
        nc.sync.dma_start(
            ones_sb[:],
            bass.AP(tensor=onesrow.tensor, offset=onesrow.offset,
                    ap=[[0, P], [1, 2]]).bitcast(f32r),
        )
        eps_sb = wp.tile([P, 1], f32)
        nc.vector.memset(eps_sb[:], EPS)

        # chunk list: (col0 in xT, width, output row or None for warmup)
        chunks = []
        if warm:
            chunks.append((0, warm, None))
        for i in range(nch):
            chunks.append((warm + i * C, C, i * C))

        prev_h = None  # (h_chunk_tile, width)
        for col0, cw, orow in chunks:
            # -- load x (pre-transposed) for this chunk
            xts = []
            for dg in range(KD):
                xt_t = xtp.tile([P, C], f32r, tag="xt", name=f"xt{dg}")
                nc.sync.dma_start(xt_t[:, :cw], xT[dg * P:(dg + 1) * P, col0:col0 + cw])
                xts.append(xt_t)

            h_ch = hp.tile([P, KH, C], f32r, name="hch")
            for hg in range(KH):
                hs = slice(hg * P, (hg + 1) * P)
                ps_ih = pproj.tile([P, C], f32, tag="pp", name="psih")
                ps_g = pproj.tile([P, C], f32, tag="pp", name="psg")
                for kd in range(KD):
                    nc.tensor.matmul(
                        ps_ih[:, :cw],
                        w_ih_sb[:, kd, hs],
                        xts[kd][:, :cw],
                        start=(kd == 0),
                        stop=(kd == KD - 1),
                    )
                for kd in range(KD):
                    nc.tensor.matmul(
                        ps_g[:, :cw],
                        w_g_sb[:, kd, hs],
                        xts[kd][:, :cw],
                        start=(kd == 0),
                        stop=(kd == KD - 1),
                    )
                ih_sb = iop.tile([P, C], f32, tag="io", name="ihsb")
                g_sb = iop.tile([P, C], f32, tag="io", name="gsb")
                # ih = psum + b_ih ; gate = sigmoid(psum + b_gate)
                nc.scalar.activation(ih_sb[:, :cw], ps_ih[:, :cw], AF.Identity,
                                     bias=vec_sb[:, 2, hg:hg + 1])
                nc.scalar.activation(g_sb[:, :cw], ps_g[:, :cw], AF.Sigmoid,
                                     bias=vec_sb[:, 3, hg:hg + 1])
                # beta = gate*ih (in place over ih); alpha = gate*(-a)+a (in place over gate)
                nc.vector.tensor_mul(ih_sb[:, :cw], g_sb[:, :cw], ih_sb[:, :cw])
                nc.vector.tensor_scalar(g_sb[:, :cw], g_sb[:, :cw],
                                        vec_sb[:, 1, hg:hg + 1], vec_sb[:, 0, hg:hg + 1],
                                        op0=OP.mult, op1=OP.add)
                init = 0.0 if prev_h is None else prev_h[0][:, hg, prev_h[1] - 1:prev_h[1]]
                nc.vector.tensor_tensor_scan(h_ch[:, hg, :cw], g_sb[:, :cw], ih_sb[:, :cw],
                                             init, op0=OP.mult, op1=OP.add)
            prev_h = (h_ch, cw)
            if orow is None:
                continue

            # -- LayerNorm stats: per t, sum and sumsq over all H via N=1 matmuls
            ps_st = pst.tile([P, 4 * TT], f32, name="psst")
            hsq = hsqp.tile([P, KH, C], f32r, tag="hsq", name="hsq")
            for hg in range(KH):
                nc.scalar.activation(hsq[:, hg, :], h_ch[:, hg, :], AF.Square)
            for ti in range(TT):
                for hg in range(KH):
                    nc.tensor.matmul(
                        ps_st[:, 4 * ti:4 * ti + 2],
                        h_ch[:, hg, ts(ti, P)],
                        ones_sb[:],
                        start=(hg == 0), stop=(hg == KH - 1),
                    )
                for hg in range(KH):
                    nc.tensor.matmul(
                        ps_st[:, 4 * ti + 2:4 * ti + 4],
                        hsq[:, hg, ts(ti, P)],
                        ones_sb[:],
                        start=(hg == 0), stop=(hg == KH - 1),
                    )
            rts = []
            for ti in range(TT):
                # cols: 0=mu 1=mu^2 2=var 3=sd 4=r 5=-r*mu
                rt_t = rp.tile([P, 6], f32, tag="r", name="rt")
                nc.scalar.mul(rt_t[:, 0:1], ps_st[:, 4 * ti:4 * ti + 1], 1.0 / H)
                nc.vector.tensor_mul(rt_t[:, 1:2], rt_t[:, 0:1], rt_t[:, 0:1])
                nc.vector.tensor_scalar(rt_t[:, 2:3], ps_st[:, 4 * ti + 2:4 * ti + 3],
                                        1.0 / H, rt_t[:, 1:2], op0=OP.mult, op1=OP.subtract)
                nc.scalar.activation(rt_t[:, 3:4], rt_t[:, 2:3], AF.Sqrt, bias=eps_sb[:])
                nc.vector.reciprocal(rt_t[:, 4:5], rt_t[:, 3:4])
                nc.vector.tensor_scalar(rt_t[:, 5:6], rt_t[:, 4:5], rt_t[:, 0:1], -1.0,
                                        op0=OP.mult, op1=OP.mult)
                rts.append(rt_t)

            # -- fused output matmul + LayerNorm application
            for ti in range(TT):
                y_sb = yp.tile([P, D], f32, tag="y", name="ysb")
                for dh in range(2):
                    ps_o = pout.tile([P, 512], f32, tag="po", name="pso")
                    for hg in range(KH):
                        nc.tensor.matmul(
                            ps_o[:],
                            h_ch[:, hg, ts(ti, P)],
                            w_c_sb[:, hg, ts(dh, 512)],
                            start=(hg == 0), stop=(hg == KH - 1),
                        )
                    ys = y_sb[:, ts(dh, 512)]
                    nc.vector.tensor_scalar(ys, ps_o[:], rts[ti][:, 4:5], None, op0=OP.mult)
                    nc.vector.scalar_tensor_tensor(ys, sbc[:, ts(dh, 512)], rts[ti][:, 5:6],
                                                   ys, op0=OP.mult, op1=OP.add)
                    nc.vector.tensor_add(ys, ys, bcb[:, ts(dh, 512)])
                nc.sync.dma_start(y[orow + ti * P: orow + (ti + 1) * P, :], y_sb[:])

    nc.compile()
    return nc


def _get_nc(rt=RT, warm=WARM, nch=SEG // C):
    key = (rt, warm, nch)
    if key not in _CACHE:
        _CACHE[key] = _build(rt, warm, nch)
    return _CACHE[key]


def _host_prep(x, W_ih, b_ih, W_gate, b_gate, log_a, ln_g, ln_b, W_out, b_out,
               W_proj, b_proj):
    """Fold weights and build per-core input shards."""
    f32 = np.float32
    x = np.asarray(x, f32)
    W_ih = np.asarray(W_ih, f32)
    W_gate = np.asarray(W_gate, f32)
    b_ih = np.asarray(b_ih, f32)
    b_gate = np.asarray(b_gate, f32)

    a = np.exp(np.asarray(log_a, np.float64))
    Wcc = np.asarray(W_out, np.float64) @ np.asarray(W_proj, np.float64)
    W_c = (np.asarray(ln_g, np.float64)[:, None] * Wcc).astype(f32)
    bc = (np.asarray(ln_b, np.float64) @ Wcc
          + np.asarray(b_out, np.float64) @ np.asarray(W_proj, np.float64)
          + np.asarray(b_proj, np.float64))
    s_row = W_c.astype(np.float64).sum(axis=0)
    vecs = np.stack([a, -a, b_ih.astype(np.float64), b_gate.astype(np.float64),
                     s_row, bc, np.ones(H)]).astype(f32)
    vecs = np.ascontiguousarray(vecs)

    in_maps = []
    for c in range(NCORES):
        b, s = c // 2, c % 2
        if s == 0:
            xs = np.concatenate([np.zeros((WARM, D), f32), x[b, :SEG]], axis=0)
        else:
            xs = x[b, SEG - WARM: T]
        xT_np = np.ascontiguousarray(xs.T)
        in_maps.append({"xT": xT_np, "wih": W_ih, "wg": W_gate,
                        "wc": W_c, "vecs": vecs})
    return in_maps


LAST_RESULTS = None


def kernel(**inputs):
    global LAST_RESULTS
    from concourse.bass_utils import run_bass_kernel_spmd

    in_maps = _host_prep(**inputs)
    nc = _get_nc()
    res = run_bass_kernel_spmd(nc, in_maps, core_ids=list(range(NCORES)))
    LAST_RESULTS = res
    y = np.empty((B, T, D), np.float32)
    for c in range(NCORES):
        b, s = c // 2, c % 2
        y[b, s * SEG:(s + 1) * SEG] = res.results[c]["y"]
    return y
